# revision 44
# baseline (speedup 1.0000x reference)
"""Trainium2 Bass kernel for nn_DecoderRNN (attention-LSTM decoder, teacher forcing).

Key mathematical simplification: the reference's attention is degenerate --
`feats` has a singleton annotation axis, so softmax over it is exactly 1.0 and
`context == features` at every step. The whole Wa/Ua/va branch cancels out.

What remains per step t:
    gates = emb_t @ W_ih[:, :E].T + features @ W_ih[:, E:].T + b_ih
            + h @ W_hh.T + b_hh
    i,f,g,o = split(gates); c = sig(f)*c + sig(i)*tanh(g); h = sig(o)*tanh(c)
    out[:, t, :] = h @ fc_W + fc_b

Sharding: the output [128, 32, 32000] fp32 (524 MB) dominates; we shard the
vocab dim of fc_W/fc_b/out across the 8 cores (4000 cols each, padded to 4096)
and replicate the (tiny) recurrence on every core. No collectives needed --
each core writes a disjoint V-slice, host concatenates.

Per-core layout (device):
  - batch B=128 lives on SBUF partitions for gates/elementwise/fc-output
  - h is transposed each step ([B,H] -> [H,B] via PE transpose) because both
    the gates matmul and the fc matmul need hT as the stationary operand
  - fc matmul: out[B, 512-slice] = hT.T @ fc_W[hchunk, slice], f32r dtype
    (1 cycle/row vs 4 for fp32)
"""

import numpy as np

import concourse.bass as bass
from concourse import bacc
import concourse.mybir as mybir
import concourse.tile as tile
from concourse.bass import ts
from concourse.bass_utils import run_bass_kernel_spmd
from concourse.masks import make_identity

B, T, V, E, H, F = 128, 32, 32000, 128, 256, 2048
NCORES = 8
VSH = V // NCORES  # 4000 true vocab slice per core
VP = 4000          # per-core fc slice width (8 x 500 chunks)
F32 = mybir.dt.float32
F32R = mybir.dt.float32r

AF = mybir.ActivationFunctionType
OP = mybir.AluOpType

_BUILT = None  # cached (nc,) so repeated kernel() calls reuse the program
LAST_RESULTS = None  # BassKernelResults of the last run (for test harness)


def _r(ap):
    return ap.bitcast(F32R)


def _build_program(trace=False):
    nc = bacc.Bacc("TRN2", target_bir_lowering=False, debug=False,
                   num_devices=NCORES)

    # ---- DRAM I/O (per-core; all cores share the program, data differs) ----
    d_embT = nc.dram_tensor("embT", [E, T * B], F32R, kind="ExternalInput").ap()
    d_wihet = nc.dram_tensor("wihet", [E, 4 * H], F32R, kind="ExternalInput").ap()
    d_whht = nc.dram_tensor("whht", [128, H // 128, 4 * H], F32R,
                            kind="ExternalInput").ap()
    d_fcw = nc.dram_tensor("fcw", [128, H // 128, VP], F32R,
                           kind="ExternalInput").ap()
    d_featc = nc.dram_tensor("featc", [B, 4 * H], F32R,
                             kind="ExternalInput").ap()
    d_h0 = nc.dram_tensor("h0", [B, H], F32, kind="ExternalInput").ap()
    d_c0 = nc.dram_tensor("c0", [B, H], F32, kind="ExternalInput").ap()
    d_fcb = nc.dram_tensor("fcb", [1, VP], F32, kind="ExternalInput").ap()
    d_out = nc.dram_tensor("out", [B, T, VP], F32, kind="ExternalOutput").ap()

    KF = F // 128   # 16 feature-dim chunks
    KH = H // 128   # 2 hidden-dim chunks

    from contextlib import ExitStack

    with tile.TileContext(nc) as tc, ExitStack() as ctx:
        const = ctx.enter_context(tc.tile_pool(name="const", bufs=1))
        state = ctx.enter_context(tc.tile_pool(name="state", bufs=2))
        work = ctx.enter_context(tc.tile_pool(name="work", bufs=2))
        pg = ctx.enter_context(tc.tile_pool(name="pg", bufs=2, space="PSUM"))
        pfc = ctx.enter_context(tc.tile_pool(name="pfc", bufs=6, space="PSUM"))

        ident = const.tile([128, 128], F32)
        make_identity(nc, ident)
        # Warm the ACT function tables (Sigmoid/Tanh/Copy) immediately so
        # their table-load DMAs enqueue before the big input loads -- else
        # step 0's activations stall ~40us behind fcw/embT in the DMA FIFO.
        warm = const.tile([128, 4], F32)
        nc.vector.memset(warm, 0.0)
        nc.scalar.activation(warm[:, 0:1], warm[:, 0:1], AF.Sigmoid)
        nc.scalar.activation(warm[:, 1:2], warm[:, 1:2], AF.Tanh)
        nc.scalar.activation(warm[:, 2:3], warm[:, 2:3], AF.Copy)
        # f32r identity (DVE copy performs the f32r rounding) -- initializes
        # gates PSUM with featc via a single matmul
        identr = const.tile([128, 128], F32R)
        nc.vector.tensor_copy(identr, ident)

        # persistent SBUF tensors, DMA-ordered by first use: the small
        # recurrence inputs first, embT (gates 0), fcw halves, fcb last
        wihet_sb = const.tile([E, 4 * H], F32R)
        nc.sync.dma_start(wihet_sb, d_wihet)
        whht_sb = const.tile([128, KH, 4 * H], F32R)
        nc.sync.dma_start(whht_sb, d_whht)
        featc_sb = const.tile([128, 4 * H], F32R)
        nc.sync.dma_start(featc_sb, d_featc)
        h_sb = state.tile([B, H], F32, tag="h")
        nc.sync.dma_start(h_sb, d_h0)
        c_sb = state.tile([B, H], F32, tag="c")
        nc.sync.dma_start(c_sb, d_c0)
        embT_sb = const.tile([E, T * B], F32R)
        nc.sync.dma_start(embT_sb, d_embT)
        fcw_sb = const.tile([128, KH, VP], F32R)
        nc.sync.dma_start(fcw_sb[:, :, 0:VP // 2], d_fcw[:, :, 0:VP // 2])
        nc.sync.dma_start(fcw_sb[:, :, VP // 2:], d_fcw[:, :, VP // 2:])
        fcb_sb = const.tile([128, VP], F32)
        nc.gpsimd.dma_start(fcb_sb, d_fcb.to_broadcast([128, VP]))

        # ---- transpose h -> hT (PSUM slots shared with the fc pool);
        # the two PSUM->SBUF copies run in parallel on DVE and ACT ----
        def transpose_h(h_in):
            hT = state.tile([128, KH, B], F32R, tag="hT")
            pts = []
            for j in range(KH):
                pt = pfc.tile([128, 128], F32, tag="fc", name=f"pt{j}")
                nc.tensor.transpose(pt, h_in[:, j * 128:(j + 1) * 128], ident)
                pts.append(pt)
            nc.vector.tensor_copy(hT[:, 0, :], pts[0])
            nc.vector.tensor_copy(hT[:, 1, :], pts[1])
            return hT

        hT = transpose_h(h_sb)

        # ---- fc half-block (2048 cols): matmuls and moves split so the
        # PSUM->SBUF moves can be issued off the recurrence-critical path ----
        FCC = VP // 8  # 500
        def fc_half_mms(hT_s, half):
            pss = []
            for i in range(4):
                ps = pfc.tile([128, FCC], F32, tag="fc", name=f"fcps{half}_{i}")
                n0 = half * (VP // 2) + i * FCC
                nc.tensor.matmul(ps, hT_s[:, 0, :],
                                 fcw_sb[:, 0, n0:n0 + FCC],
                                 start=True, stop=False)
                nc.tensor.matmul(ps, hT_s[:, 1, :],
                                 fcw_sb[:, 1, n0:n0 + FCC],
                                 start=False, stop=True)
                pss.append(ps)
            return pss

        def fc_half_moves(s, half, pss, on_act):
            # priority-shifted so the scheduler never interleaves these
            # PSUM->SBUF moves into the recurrence-critical ACT/DVE chain
            with tc.high_priority(offset=-100):
                out_sb = work.tile([B, VP // 2], F32, tag=f"fco{half}")
                for i in range(4):
                    n0 = half * (VP // 2) + i * FCC
                    osl = out_sb[:, i * FCC:(i + 1) * FCC]
                    if i in on_act:
                        nc.scalar.activation(osl, pss[i], AF.Copy)
                        nc.gpsimd.tensor_tensor(osl, osl,
                                                fcb_sb[:, n0:n0 + FCC], OP.add)
                    else:
                        nc.vector.tensor_add(osl, pss[i],
                                             fcb_sb[:, n0:n0 + FCC])
                nc.sync.dma_start(
                    d_out[:, s, half * (VP // 2):(half + 1) * (VP // 2)],
                    out_sb)

        # ---- the 32 recurrence steps, software-pipelined ----
        # Gate order is host-permuted to [o, g | f, i] (half0 = o,g;
        # half1 = f,i) so tanh(g) can issue right after half0's matmuls.
        # Iteration t works on step t's elementwise chain; the PE queue per
        # iter is [transpose(t), gates-matmuls(t+1), fc(t-1,h1), fc(t,h0)]
        # so the recurrence-critical matmuls never sit behind fc matmuls.
        def emit_gates_mms(t, hT_t):
            g_halves = []
            for hh in range(2):
                hs = slice(hh * 512, (hh + 1) * 512)
                g_ps = pg.tile([128, 512], F32, tag="g", name=f"g_ps{t}_{hh}")
                nc.tensor.matmul(g_ps, identr, featc_sb[:, hs],
                                 start=True, stop=False)
                nc.tensor.matmul(g_ps, embT_sb[:, ts(t, B)],
                                 wihet_sb[:, hs], start=False, stop=False)
                nc.tensor.matmul(g_ps, hT_t[:, 0, :],
                                 whht_sb[:, 0, hs], start=False, stop=False)
                nc.tensor.matmul(g_ps, hT_t[:, 1, :],
                                 whht_sb[:, 1, hs], start=False, stop=True)
                g_halves.append(g_ps)
            return g_halves

        g_halves = emit_gates_mms(0, hT)
        hT_prev = None
        for t in range(T):
            crit = tc.high_priority(offset=None)
            crit.__enter__()
            gates_sb = work.tile([B, 4 * H], F32, tag="gates")
            # activations out of PSUM, gate order [g, i | f, o]:
            # tanh(g) and sig(i) first (both in half0, feed i*g), then f, o
            nc.scalar.activation(gates_sb[:, 0:H], g_halves[0][:, 0:H],
                                 AF.Tanh)
            nc.scalar.activation(gates_sb[:, H:2 * H], g_halves[0][:, H:2 * H],
                                 AF.Sigmoid)
            # c = f*c + i*g ; h = o*tanh(c); i*g first (inputs in half0)
            ig = work.tile([B, H], F32, tag="ig")
            nc.vector.tensor_mul(ig, gates_sb[:, H:2 * H], gates_sb[:, 0:H])
            nc.scalar.activation(gates_sb[:, 2 * H:3 * H],
                                 g_halves[1][:, 0:H], AF.Sigmoid)
            nc.scalar.activation(gates_sb[:, 3 * H:4 * H],
                                 g_halves[1][:, H:2 * H], AF.Sigmoid)
            c_new = state.tile([B, H], F32, tag="c")
            nc.vector.tensor_mul(c_new, gates_sb[:, 2 * H:3 * H], c_sb)
            nc.vector.tensor_add(c_new, c_new, ig)
            tanh_c = work.tile([B, H], F32, tag="th")
            nc.scalar.activation(tanh_c, c_new, AF.Tanh)
            h_new = state.tile([B, H], F32, tag="h")
            nc.vector.tensor_mul(h_new, gates_sb[:, 3 * H:4 * H], tanh_c)
            c_sb = c_new

            hT_pp = hT_prev
            hT_prev = hT
            hT = transpose_h(h_new)
            if t + 1 < T:
                g_halves = emit_gates_mms(t + 1, hT)
            crit.__exit__(None, None, None)

            # fc matmuls + moves, all off the critical path:
            # half1 of step t-1 (old hT), then half0 of step t (new hT)
            if t > 0:
                pss_h1 = fc_half_mms(hT_prev, 1)
                fc_half_moves(t - 1, 1, pss_h1, on_act=(0, 1, 2, 3))
            pss_h0 = fc_half_mms(hT, 0)
            fc_half_moves(t, 0, pss_h0, on_act=())

        pss_h1 = fc_half_mms(hT, 1)
        fc_half_moves(T - 1, 1, pss_h1, on_act=(0, 1, 2, 3))

    nc.compile()
    return nc


def prepare_in_maps(inputs):
    i = {k: np.asarray(v) for k, v in inputs.items()}

    features = np.ascontiguousarray(i["features"], dtype=np.float32)
    captions = np.asarray(i["captions"]).astype(np.int64)
    embeddings = np.asarray(i["embeddings"], dtype=np.float32)
    W_ih = np.asarray(i["W_ih"], dtype=np.float32)
    W_hh = np.asarray(i["W_hh"], dtype=np.float32)
    fc_W = np.asarray(i["fc_W"], dtype=np.float32)

    # host-side input prep: sharding, layout transposes, the embedding
    # gather, and the three tiny time-invariant GEMMs (featc, h0, c0 --
    # ~1% of total FLOPs; they would otherwise serialize 13MB of weight
    # DMA in front of step 0 on every core).
    # Gate order is permuted from PyTorch [i,f,g,o] to [g,i,f,o] so the
    # device's activation order matches the dependency chain (tanh(g) and
    # sig(i) first from PSUM half0 -> i*g; f next -> f*c; o last).
    perm = np.r_[np.arange(2 * H, 3 * H), np.arange(0, H),
                 np.arange(H, 2 * H), np.arange(3 * H, 4 * H)]
    W_ih = W_ih[perm]
    W_hh = W_hh[perm]
    emb = embeddings[captions]                      # [B, T, E] gather
    embT = np.ascontiguousarray(emb.transpose(2, 1, 0)).reshape(E, T * B)
    wihet = np.ascontiguousarray(W_ih[:, :E].T)     # [E, 4H]
    whht = np.ascontiguousarray(
        W_hh.T.reshape(H // 128, 128, 4 * H).transpose(1, 0, 2))
    bias_g = (np.asarray(i["b_ih"]) + np.asarray(i["b_hh"]))[perm]
    featc = (features @ W_ih[:, E:].T + bias_g).astype(np.float32)  # [B, 4H]
    h0 = (features @ np.asarray(i["initH_W"], np.float32)
          + np.asarray(i["initH_b"], np.float32)).astype(np.float32)
    c0 = (features @ np.asarray(i["initC_W"], np.float32)
          + np.asarray(i["initC_b"], np.float32)).astype(np.float32)

    in_maps = []
    common = {
        "embT": embT, "wihet": wihet, "whht": whht,
        "featc": featc, "h0": h0, "c0": c0,
    }
    for ci in range(NCORES):
        fcw_c = np.ascontiguousarray(
            fc_W[:, ci * VSH:(ci + 1) * VSH]
            .reshape(H // 128, 128, VP).transpose(1, 0, 2))
        fcb_c = np.ascontiguousarray(
            np.asarray(i["fc_b"], np.float32)[ci * VSH:(ci + 1) * VSH]
            .reshape(1, VP))
        m = dict(common)
        m["fcw"] = fcw_c
        m["fcb"] = fcb_c
        in_maps.append(m)
    return in_maps


def kernel(**inputs):
    global _BUILT, LAST_RESULTS
    in_maps = prepare_in_maps(inputs)

    if _BUILT is None:
        _BUILT = _build_program()
    nc = _BUILT

    res = run_bass_kernel_spmd(nc, in_maps, core_ids=list(range(NCORES)),
                               trace=bool(int(__import__("os").environ.get(
                                   "KERNEL_TRACE", "0"))))
    LAST_RESULTS = res

    out = np.concatenate(
        [res.results[ci]["out"] for ci in range(NCORES)], axis=2)
    return out


# revision 45
# speedup vs baseline: 341.7936x; 341.7936x over previous
"""Trainium2 Bass kernel for nn_DecoderRNN (attention-LSTM decoder, teacher forcing).

Key mathematical simplification: the reference's attention is degenerate --
`feats` has a singleton annotation axis, so softmax over it is exactly 1.0 and
`context == features` at every step. The whole Wa/Ua/va branch cancels out.

What remains per step t:
    gates = emb_t @ W_ih[:, :E].T + features @ W_ih[:, E:].T + b_ih
            + h @ W_hh.T + b_hh
    i,f,g,o = split(gates); c = sig(f)*c + sig(i)*tanh(g); h = sig(o)*tanh(c)
    out[:, t, :] = h @ fc_W + fc_b

Sharding: the output [128, 32, 32000] fp32 (524 MB) dominates; we shard the
vocab dim of fc_W/fc_b/out across the 8 cores (4000 cols each, padded to 4096)
and replicate the (tiny) recurrence on every core. No collectives needed --
each core writes a disjoint V-slice, host concatenates.

Per-core layout (device):
  - batch B=128 lives on SBUF partitions for gates/elementwise/fc-output
  - h is transposed each step ([B,H] -> [H,B] via PE transpose) because both
    the gates matmul and the fc matmul need hT as the stationary operand
  - fc matmul: out[B, 512-slice] = hT.T @ fc_W[hchunk, slice], f32r dtype
    (1 cycle/row vs 4 for fp32)
"""

import numpy as np

import concourse.bass as bass
from concourse import bacc
import concourse.mybir as mybir
import concourse.tile as tile
from concourse.bass import ts
from concourse.bass_utils import run_bass_kernel_spmd
from concourse.masks import make_identity

B, T, V, E, H, F = 128, 32, 32000, 128, 256, 2048
NCORES = 8
VSH = V // NCORES  # 4000 true vocab slice per core
VP = 4000          # per-core fc slice width (8 x 500 chunks)
F32 = mybir.dt.float32
F32R = mybir.dt.float32r

AF = mybir.ActivationFunctionType
OP = mybir.AluOpType

_BUILT = None  # cached (nc,) so repeated kernel() calls reuse the program
LAST_RESULTS = None  # BassKernelResults of the last run (for test harness)


def _build_program(trace=False):
    nc = bacc.Bacc("TRN2", target_bir_lowering=False, debug=False,
                   num_devices=NCORES)

    # ---- DRAM I/O (per-core; all cores share the program, data differs) ----
    d_embT = nc.dram_tensor("embT", [E, T * B], F32R, kind="ExternalInput").ap()
    d_wihet = nc.dram_tensor("wihet", [E, 4 * H], F32R, kind="ExternalInput").ap()
    d_whht = nc.dram_tensor("whht", [128, H // 128, 4 * H], F32R,
                            kind="ExternalInput").ap()
    d_fcw = nc.dram_tensor("fcw", [128, H // 128, VP], F32R,
                           kind="ExternalInput").ap()
    d_featc = nc.dram_tensor("featc", [B, 4 * H], F32R,
                             kind="ExternalInput").ap()
    d_h0 = nc.dram_tensor("h0", [B, H], F32, kind="ExternalInput").ap()
    d_c0 = nc.dram_tensor("c0", [B, H], F32, kind="ExternalInput").ap()
    d_fcb = nc.dram_tensor("fcb", [1, VP], F32, kind="ExternalInput").ap()
    d_out = nc.dram_tensor("out", [B, T, VP], F32, kind="ExternalOutput").ap()

    KF = F // 128   # 16 feature-dim chunks
    KH = H // 128   # 2 hidden-dim chunks

    from contextlib import ExitStack

    with tile.TileContext(nc) as tc, ExitStack() as ctx:
        const = ctx.enter_context(tc.tile_pool(name="const", bufs=1))
        state = ctx.enter_context(tc.tile_pool(name="state", bufs=2))
        work = ctx.enter_context(tc.tile_pool(name="work", bufs=2))
        pg = ctx.enter_context(tc.tile_pool(name="pg", bufs=2, space="PSUM"))
        pfc = ctx.enter_context(tc.tile_pool(name="pfc", bufs=6, space="PSUM"))

        ident = const.tile([128, 128], F32)
        make_identity(nc, ident)
        # Warm the ACT function tables (Sigmoid/Tanh/Copy) immediately so
        # their table-load DMAs enqueue before the big input loads -- else
        # step 0's activations stall ~40us behind fcw/embT in the DMA FIFO.
        warm = const.tile([128, 4], F32)
        nc.vector.memset(warm, 0.0)
        nc.scalar.activation(warm[:, 0:1], warm[:, 0:1], AF.Sigmoid)
        nc.scalar.activation(warm[:, 1:2], warm[:, 1:2], AF.Tanh)
        nc.scalar.activation(warm[:, 2:3], warm[:, 2:3], AF.Copy)
        # f32r identity (DVE copy performs the f32r rounding) -- initializes
        # gates PSUM with featc via a single matmul
        identr = const.tile([128, 128], F32R)
        nc.vector.tensor_copy(identr, ident)

        # persistent SBUF tensors, DMA-ordered by first use: the small
        # recurrence inputs first, embT (gates 0), fcw halves, fcb last
        wihet_sb = const.tile([E, 4 * H], F32R)
        nc.sync.dma_start(wihet_sb, d_wihet)
        whht_sb = const.tile([128, KH, 4 * H], F32R)
        nc.sync.dma_start(whht_sb, d_whht)
        featc_sb = const.tile([128, 4 * H], F32R)
        nc.sync.dma_start(featc_sb, d_featc)
        h_sb = state.tile([B, H], F32, tag="h")
        nc.sync.dma_start(h_sb, d_h0)
        c_sb = state.tile([B, H], F32, tag="c")
        nc.sync.dma_start(c_sb, d_c0)
        embT_sb = const.tile([E, T * B], F32R)
        nc.sync.dma_start(embT_sb, d_embT)
        fcw_sb = const.tile([128, KH, VP], F32R)
        nc.sync.dma_start(fcw_sb[:, :, 0:VP // 2], d_fcw[:, :, 0:VP // 2])
        nc.sync.dma_start(fcw_sb[:, :, VP // 2:], d_fcw[:, :, VP // 2:])
        fcb_sb = const.tile([128, VP], F32)
        nc.gpsimd.dma_start(fcb_sb, d_fcb.to_broadcast([128, VP]))

        # ---- transpose h -> hT (PSUM slots shared with the fc pool);
        # the two PSUM->SBUF copies run in parallel on DVE and ACT ----
        def transpose_h(h_in):
            hT = state.tile([128, KH, B], F32R, tag="hT")
            pts = []
            for j in range(KH):
                pt = pfc.tile([128, 128], F32, tag="fc", name=f"pt{j}")
                nc.tensor.transpose(pt, h_in[:, j * 128:(j + 1) * 128], ident)
                pts.append(pt)
            nc.vector.tensor_copy(hT[:, 0, :], pts[0])
            nc.vector.tensor_copy(hT[:, 1, :], pts[1])
            return hT

        hT = transpose_h(h_sb)

        # ---- fc half-block (2048 cols): matmuls and moves split so the
        # PSUM->SBUF moves can be issued off the recurrence-critical path ----
        FCC = VP // 8  # 500
        def fc_half_mms(hT_s, half):
            pss = []
            for i in range(4):
                ps = pfc.tile([128, FCC], F32, tag="fc", name=f"fcps{half}_{i}")
                n0 = half * (VP // 2) + i * FCC
                nc.tensor.matmul(ps, hT_s[:, 0, :],
                                 fcw_sb[:, 0, n0:n0 + FCC],
                                 start=True, stop=False)
                nc.tensor.matmul(ps, hT_s[:, 1, :],
                                 fcw_sb[:, 1, n0:n0 + FCC],
                                 start=False, stop=True)
                pss.append(ps)
            return pss

        def fc_half_moves(s, half, pss, on_act):
            # priority-shifted so the scheduler never interleaves these
            # PSUM->SBUF moves into the recurrence-critical ACT/DVE chain
            with tc.high_priority(offset=-100):
                out_sb = work.tile([B, VP // 2], F32, tag=f"fco{half}")
                for i in range(4):
                    n0 = half * (VP // 2) + i * FCC
                    osl = out_sb[:, i * FCC:(i + 1) * FCC]
                    if i in on_act:
                        nc.scalar.activation(osl, pss[i], AF.Copy)
                        nc.gpsimd.tensor_tensor(osl, osl,
                                                fcb_sb[:, n0:n0 + FCC], OP.add)
                    else:
                        nc.vector.tensor_add(osl, pss[i],
                                             fcb_sb[:, n0:n0 + FCC])
                nc.sync.dma_start(
                    d_out[:, s, half * (VP // 2):(half + 1) * (VP // 2)],
                    out_sb)

        # ---- the 32 recurrence steps, software-pipelined ----
        # Gate order is host-permuted to [o, g | f, i] (half0 = o,g;
        # half1 = f,i) so tanh(g) can issue right after half0's matmuls.
        # Iteration t works on step t's elementwise chain; the PE queue per
        # iter is [transpose(t), gates-matmuls(t+1), fc(t-1,h1), fc(t,h0)]
        # so the recurrence-critical matmuls never sit behind fc matmuls.
        def emit_gates_mms(t, hT_t):
            g_halves = []
            for hh in range(2):
                hs = slice(hh * 512, (hh + 1) * 512)
                g_ps = pg.tile([128, 512], F32, tag="g", name=f"g_ps{t}_{hh}")
                nc.tensor.matmul(g_ps, identr, featc_sb[:, hs],
                                 start=True, stop=False)
                nc.tensor.matmul(g_ps, embT_sb[:, ts(t, B)],
                                 wihet_sb[:, hs], start=False, stop=False)
                nc.tensor.matmul(g_ps, hT_t[:, 0, :],
                                 whht_sb[:, 0, hs], start=False, stop=False)
                nc.tensor.matmul(g_ps, hT_t[:, 1, :],
                                 whht_sb[:, 1, hs], start=False, stop=True)
                g_halves.append(g_ps)
            return g_halves

        g_halves = emit_gates_mms(0, hT)
        hT_prev = None
        for t in range(T):
            crit = tc.high_priority(offset=None)
            crit.__enter__()
            gates_sb = work.tile([B, 4 * H], F32, tag="gates")
            # activations out of PSUM, gate order [g, i | f, o]:
            # tanh(g) and sig(i) first (both in half0, feed i*g), then f, o
            nc.scalar.activation(gates_sb[:, 0:H], g_halves[0][:, 0:H],
                                 AF.Tanh)
            nc.scalar.activation(gates_sb[:, H:2 * H], g_halves[0][:, H:2 * H],
                                 AF.Sigmoid)
            # c = f*c + i*g ; h = o*tanh(c); i*g first (inputs in half0)
            ig = work.tile([B, H], F32, tag="ig")
            nc.vector.tensor_mul(ig, gates_sb[:, H:2 * H], gates_sb[:, 0:H])
            nc.scalar.activation(gates_sb[:, 2 * H:3 * H],
                                 g_halves[1][:, 0:H], AF.Sigmoid)
            nc.scalar.activation(gates_sb[:, 3 * H:4 * H],
                                 g_halves[1][:, H:2 * H], AF.Sigmoid)
            c_new = state.tile([B, H], F32, tag="c")
            nc.vector.tensor_mul(c_new, gates_sb[:, 2 * H:3 * H], c_sb)
            nc.vector.tensor_add(c_new, c_new, ig)
            tanh_c = work.tile([B, H], F32, tag="th")
            nc.scalar.activation(tanh_c, c_new, AF.Tanh)
            h_new = state.tile([B, H], F32, tag="h")
            nc.vector.tensor_mul(h_new, gates_sb[:, 3 * H:4 * H], tanh_c)
            c_sb = c_new

            hT_prev = hT
            hT = transpose_h(h_new)
            if t + 1 < T:
                g_halves = emit_gates_mms(t + 1, hT)
            crit.__exit__(None, None, None)

            # fc matmuls + moves, all off the critical path:
            # half1 of step t-1 (old hT), then half0 of step t (new hT)
            if t > 0:
                pss_h1 = fc_half_mms(hT_prev, 1)
                fc_half_moves(t - 1, 1, pss_h1, on_act=(0, 1, 2, 3))
            pss_h0 = fc_half_mms(hT, 0)
            fc_half_moves(t, 0, pss_h0, on_act=())

        pss_h1 = fc_half_mms(hT, 1)
        fc_half_moves(T - 1, 1, pss_h1, on_act=(0, 1, 2, 3))

    nc.compile()
    return nc


def prepare_in_maps(inputs):
    i = {k: np.asarray(v) for k, v in inputs.items()}

    features = np.ascontiguousarray(i["features"], dtype=np.float32)
    captions = np.asarray(i["captions"]).astype(np.int64)
    embeddings = np.asarray(i["embeddings"], dtype=np.float32)
    W_ih = np.asarray(i["W_ih"], dtype=np.float32)
    W_hh = np.asarray(i["W_hh"], dtype=np.float32)
    fc_W = np.asarray(i["fc_W"], dtype=np.float32)

    # host-side input prep: sharding, layout transposes, the embedding
    # gather, and the three tiny time-invariant GEMMs (featc, h0, c0 --
    # ~1% of total FLOPs; they would otherwise serialize 13MB of weight
    # DMA in front of step 0 on every core).
    # Gate order is permuted from PyTorch [i,f,g,o] to [g,i,f,o] so the
    # device's activation order matches the dependency chain (tanh(g) and
    # sig(i) first from PSUM half0 -> i*g; f next -> f*c; o last).
    perm = np.r_[np.arange(2 * H, 3 * H), np.arange(0, H),
                 np.arange(H, 2 * H), np.arange(3 * H, 4 * H)]
    W_ih = W_ih[perm]
    W_hh = W_hh[perm]
    emb = embeddings[captions]                      # [B, T, E] gather
    embT = np.ascontiguousarray(emb.transpose(2, 1, 0)).reshape(E, T * B)
    wihet = np.ascontiguousarray(W_ih[:, :E].T)     # [E, 4H]
    whht = np.ascontiguousarray(
        W_hh.T.reshape(H // 128, 128, 4 * H).transpose(1, 0, 2))
    bias_g = (np.asarray(i["b_ih"]) + np.asarray(i["b_hh"]))[perm]
    featc = (features @ W_ih[:, E:].T + bias_g).astype(np.float32)  # [B, 4H]
    h0 = (features @ np.asarray(i["initH_W"], np.float32)
          + np.asarray(i["initH_b"], np.float32)).astype(np.float32)
    c0 = (features @ np.asarray(i["initC_W"], np.float32)
          + np.asarray(i["initC_b"], np.float32)).astype(np.float32)

    in_maps = []
    common = {
        "embT": embT, "wihet": wihet, "whht": whht,
        "featc": featc, "h0": h0, "c0": c0,
    }
    for ci in range(NCORES):
        fcw_c = np.ascontiguousarray(
            fc_W[:, ci * VSH:(ci + 1) * VSH]
            .reshape(H // 128, 128, VP).transpose(1, 0, 2))
        fcb_c = np.ascontiguousarray(
            np.asarray(i["fc_b"], np.float32)[ci * VSH:(ci + 1) * VSH]
            .reshape(1, VP))
        m = dict(common)
        m["fcw"] = fcw_c
        m["fcb"] = fcb_c
        in_maps.append(m)
    return in_maps


def kernel(**inputs):
    global _BUILT, LAST_RESULTS
    in_maps = prepare_in_maps(inputs)

    if _BUILT is None:
        _BUILT = _build_program()
    nc = _BUILT

    res = run_bass_kernel_spmd(nc, in_maps, core_ids=list(range(NCORES)),
                               trace=bool(int(__import__("os").environ.get(
                                   "KERNEL_TRACE", "0"))))
    LAST_RESULTS = res

    out = np.concatenate(
        [res.results[ci]["out"] for ci in range(NCORES)], axis=2)
    return out


# revision 51
# speedup vs baseline: 348.5385x; 1.0197x over previous
"""Trainium2 Bass kernel for nn_DecoderRNN (attention-LSTM decoder, teacher forcing).

Key mathematical simplification: the reference's attention is degenerate --
`feats` has a singleton annotation axis, so softmax over it is exactly 1.0 and
`context == features` at every step. The whole Wa/Ua/va branch cancels out.

What remains per step t:
    gates = emb_t @ W_ih[:, :E].T + features @ W_ih[:, E:].T + b_ih
            + h @ W_hh.T + b_hh
    i,f,g,o = split(gates); c = sig(f)*c + sig(i)*tanh(g); h = sig(o)*tanh(c)
    out[:, t, :] = h @ fc_W + fc_b

Sharding: the output [128, 32, 32000] fp32 (524 MB) dominates; we shard the
vocab dim of fc_W/fc_b/out across the 8 cores (4000 cols each, padded to 4096)
and replicate the (tiny) recurrence on every core. No collectives needed --
each core writes a disjoint V-slice, host concatenates.

Per-core layout (device):
  - batch B=128 lives on SBUF partitions for gates/elementwise/fc-output
  - h is transposed each step ([B,H] -> [H,B] via PE transpose) because both
    the gates matmul and the fc matmul need hT as the stationary operand
  - fc matmul: out[B, 512-slice] = hT.T @ fc_W[hchunk, slice], f32r dtype
    (1 cycle/row vs 4 for fp32)
"""

import numpy as np

import concourse.bass as bass
from concourse import bacc
import concourse.mybir as mybir
import concourse.tile as tile
from concourse.bass import ts
from concourse.bass_utils import run_bass_kernel_spmd
from concourse.masks import make_identity

B, T, V, E, H, F = 128, 32, 32000, 128, 256, 2048
NCORES = 8
VSH = V // NCORES  # 4000 true vocab slice per core
VP = 4000          # per-core fc slice width (8 x 500 chunks)
F32 = mybir.dt.float32
F32R = mybir.dt.float32r

AF = mybir.ActivationFunctionType
OP = mybir.AluOpType

_BUILT = None  # cached (nc,) so repeated kernel() calls reuse the program
LAST_RESULTS = None  # BassKernelResults of the last run (for test harness)


KH0 = H // 128


def _build_program(trace=False):
    nc = bacc.Bacc("TRN2", target_bir_lowering=False, debug=False,
                   num_devices=NCORES)

    # ---- DRAM I/O (per-core; all cores share the program, data differs) ----
    d_embT = nc.dram_tensor("embT", [E, T * B], F32R, kind="ExternalInput").ap()
    d_packr = nc.dram_tensor("packr", [128, 4 * H + KH0 * 4 * H + 4 * H], F32R,
                             kind="ExternalInput").ap()
    d_fcw = nc.dram_tensor("fcw", [128, H // 128, VP], F32R,
                           kind="ExternalInput").ap()
    d_packf = nc.dram_tensor("packf", [B, 2 * H], F32, kind="ExternalInput").ap()
    d_fcb = nc.dram_tensor("fcb", [1, VP], F32, kind="ExternalInput").ap()
    d_out = nc.dram_tensor("out", [B, T, VP], F32, kind="ExternalOutput").ap()

    KF = F // 128   # 16 feature-dim chunks
    KH = H // 128   # 2 hidden-dim chunks

    from contextlib import ExitStack

    with tile.TileContext(nc) as tc, ExitStack() as ctx:
        const = ctx.enter_context(tc.tile_pool(name="const", bufs=1))
        state = ctx.enter_context(tc.tile_pool(name="state", bufs=2))
        work = ctx.enter_context(tc.tile_pool(name="work", bufs=2))
        pg = ctx.enter_context(tc.tile_pool(name="pg", bufs=2, space="PSUM"))
        pfc = ctx.enter_context(tc.tile_pool(name="pfc", bufs=6, space="PSUM"))

        ident = const.tile([128, 128], F32)
        make_identity(nc, ident)
        # Warm the ACT function tables (Sigmoid/Tanh/Copy) immediately so
        # their table-load DMAs enqueue before the big input loads -- else
        # step 0's activations stall ~40us behind fcw/embT in the DMA FIFO.
        warm = const.tile([128, 4], F32)
        nc.vector.memset(warm, 0.0)
        nc.scalar.activation(warm[:, 0:1], warm[:, 0:1], AF.Sigmoid)
        nc.scalar.activation(warm[:, 1:2], warm[:, 1:2], AF.Tanh)
        nc.scalar.activation(warm[:, 2:3], warm[:, 2:3], AF.Copy)
        # f32r identity (DVE copy performs the f32r rounding) -- initializes
        # gates PSUM with featc via a single matmul
        identr = const.tile([128, 128], F32R)
        nc.vector.tensor_copy(identr, ident)

        # persistent SBUF tensors. The small recurrence inputs are packed
        # into one [128, 4096] f32r tensor (wihet | whht | featc) and one
        # [128, 512] f32 tensor (h0 | c0) -- each dma_start carries ~1.3us
        # of fixed latency, so fewer/bigger input DMAs shorten the ramp.
        packr_sb = const.tile([128, 4 * H + KH * 4 * H + 4 * H], F32R)
        nc.sync.dma_start(packr_sb, d_packr)
        wihet_sb = packr_sb[:, 0:4 * H]
        def whht_slice(j, hs):
            base = 4 * H + j * 4 * H
            return packr_sb[:, base + hs.start:base + hs.stop]
        featc_sb = packr_sb[:, (1 + KH) * 4 * H:(2 + KH) * 4 * H]
        packf_sb = state.tile([B, 2 * H], F32, tag="packf")
        nc.sync.dma_start(packf_sb, d_packf)
        h_sb = packf_sb[:, 0:H]
        c_sb = packf_sb[:, H:2 * H]
        embT_sb = const.tile([E, T * B], F32R)
        nc.sync.dma_start(embT_sb, d_embT)
        fcw_sb = const.tile([128, KH, VP], F32R)
        nc.sync.dma_start(fcw_sb[:, :, 0:VP // 2], d_fcw[:, :, 0:VP // 2])
        nc.sync.dma_start(fcw_sb[:, :, VP // 2:], d_fcw[:, :, VP // 2:])
        fcb_sb = const.tile([128, VP], F32)
        nc.gpsimd.dma_start(fcb_sb, d_fcb.to_broadcast([128, VP]))

        # ---- transpose h -> hT (PSUM slots shared with the fc pool);
        # the two PSUM->SBUF copies run in parallel on DVE and ACT ----
        def transpose_h(h_in):
            hT = state.tile([128, KH, B], F32R, tag="hT", bufs=3)
            pts = []
            for j in range(KH):
                pt = pfc.tile([128, 128], F32, tag="fc", name=f"pt{j}")
                nc.tensor.transpose(pt, h_in[:, j * 128:(j + 1) * 128], ident)
                pts.append(pt)
            nc.vector.tensor_copy(hT[:, 0, :], pts[0])
            nc.vector.tensor_copy(hT[:, 1, :], pts[1])
            return hT

        hT = transpose_h(h_sb)

        # ---- fc half-block (2048 cols): matmuls and moves split so the
        # PSUM->SBUF moves can be issued off the recurrence-critical path ----
        FCC = VP // 8  # 500
        def fc_half_mms(hT_s, half):
            pss = []
            for i in range(4):
                ps = pfc.tile([128, FCC], F32, tag="fc", name=f"fcps{half}_{i}")
                n0 = half * (VP // 2) + i * FCC
                nc.tensor.matmul(ps, hT_s[:, 0, :],
                                 fcw_sb[:, 0, n0:n0 + FCC],
                                 start=True, stop=False)
                nc.tensor.matmul(ps, hT_s[:, 1, :],
                                 fcw_sb[:, 1, n0:n0 + FCC],
                                 start=False, stop=True)
                pss.append(ps)
            return pss

        def fc_half_moves(s, half, pss, on_act):
            # priority-shifted so the scheduler never interleaves these
            # PSUM->SBUF moves into the recurrence-critical ACT/DVE chain
            with tc.high_priority(offset=-100):
                out_sb = work.tile([B, VP // 2], F32, tag=f"fco{half}", bufs=3)
                for i in range(4):
                    n0 = half * (VP // 2) + i * FCC
                    osl = out_sb[:, i * FCC:(i + 1) * FCC]
                    if i in on_act:
                        nc.scalar.activation(osl, pss[i], AF.Copy)
                        nc.gpsimd.tensor_tensor(osl, osl,
                                                fcb_sb[:, n0:n0 + FCC], OP.add)
                    else:
                        nc.vector.tensor_add(osl, pss[i],
                                             fcb_sb[:, n0:n0 + FCC])
                nc.sync.dma_start(
                    d_out[:, s, half * (VP // 2):(half + 1) * (VP // 2)],
                    out_sb)

        # ---- the 32 recurrence steps, software-pipelined ----
        # Gate order is host-permuted to [o, g | f, i] (half0 = o,g;
        # half1 = f,i) so tanh(g) can issue right after half0's matmuls.
        # Iteration t works on step t's elementwise chain; the PE queue per
        # iter is [transpose(t), gates-matmuls(t+1), fc(t-1,h1), fc(t,h0)]
        # so the recurrence-critical matmuls never sit behind fc matmuls.
        def emit_gates_mms(t, hT_t):
            g_halves = []
            for hh in range(2):
                hs = slice(hh * 512, (hh + 1) * 512)
                g_ps = pg.tile([128, 512], F32, tag="g", name=f"g_ps{t}_{hh}")
                nc.tensor.matmul(g_ps, identr, featc_sb[:, hs],
                                 start=True, stop=False)
                nc.tensor.matmul(g_ps, embT_sb[:, ts(t, B)],
                                 wihet_sb[:, hs], start=False, stop=False)
                nc.tensor.matmul(g_ps, hT_t[:, 0, :],
                                 whht_slice(0, hs), start=False, stop=False)
                nc.tensor.matmul(g_ps, hT_t[:, 1, :],
                                 whht_slice(1, hs), start=False, stop=True)
                g_halves.append(g_ps)
            return g_halves

        g_halves = emit_gates_mms(0, hT)
        hT_prev = None
        for t in range(T):
            crit = tc.high_priority(offset=None)
            crit.__enter__()
            gates_sb = work.tile([B, 4 * H], F32, tag="gates", bufs=3)
            # activations out of PSUM, gate order [g, i | f, o]:
            # tanh(g) and sig(i) first (both in half0, feed i*g), then f, o
            nc.scalar.activation(gates_sb[:, 0:H], g_halves[0][:, 0:H],
                                 AF.Tanh)
            nc.scalar.activation(gates_sb[:, H:2 * H], g_halves[0][:, H:2 * H],
                                 AF.Sigmoid)
            # c = f*c + i*g ; h = o*tanh(c); i*g first (inputs in half0)
            ig = work.tile([B, H], F32, tag="ig", bufs=3)
            nc.vector.tensor_mul(ig, gates_sb[:, H:2 * H], gates_sb[:, 0:H])
            nc.scalar.activation(gates_sb[:, 2 * H:3 * H],
                                 g_halves[1][:, 0:H], AF.Sigmoid)
            nc.scalar.activation(gates_sb[:, 3 * H:4 * H],
                                 g_halves[1][:, H:2 * H], AF.Sigmoid)
            c_new = state.tile([B, H], F32, tag="c", bufs=3)
            nc.vector.tensor_mul(c_new, gates_sb[:, 2 * H:3 * H], c_sb)
            nc.vector.tensor_add(c_new, c_new, ig)
            tanh_c = work.tile([B, H], F32, tag="th", bufs=3)
            nc.scalar.activation(tanh_c, c_new, AF.Tanh)
            h_new = state.tile([B, H], F32, tag="h", bufs=3)
            nc.vector.tensor_mul(h_new, gates_sb[:, 3 * H:4 * H], tanh_c)
            c_sb = c_new

            hT_prev = hT
            hT = transpose_h(h_new)
            if t + 1 < T:
                g_halves = emit_gates_mms(t + 1, hT)
            crit.__exit__(None, None, None)

            # fc matmuls + moves, all off the critical path:
            # half1 of step t-1 (old hT), then half0 of step t (new hT)
            if t > 0:
                pss_h1 = fc_half_mms(hT_prev, 1)
                fc_half_moves(t - 1, 1, pss_h1, on_act=(0, 1, 2, 3))
            pss_h0 = fc_half_mms(hT, 0)
            fc_half_moves(t, 0, pss_h0, on_act=())

        pss_h1 = fc_half_mms(hT, 1)
        fc_half_moves(T - 1, 1, pss_h1, on_act=(0, 1, 2, 3))

    nc.compile()
    return nc


def prepare_in_maps(inputs):
    i = {k: np.asarray(v) for k, v in inputs.items()}

    features = np.ascontiguousarray(i["features"], dtype=np.float32)
    captions = np.asarray(i["captions"]).astype(np.int64)
    embeddings = np.asarray(i["embeddings"], dtype=np.float32)
    W_ih = np.asarray(i["W_ih"], dtype=np.float32)
    W_hh = np.asarray(i["W_hh"], dtype=np.float32)
    fc_W = np.asarray(i["fc_W"], dtype=np.float32)

    # host-side input prep: sharding, layout transposes, the embedding
    # gather, and the three tiny time-invariant GEMMs (featc, h0, c0 --
    # ~1% of total FLOPs; they would otherwise serialize 13MB of weight
    # DMA in front of step 0 on every core).
    # Gate order is permuted from PyTorch [i,f,g,o] to [g,i,f,o] so the
    # device's activation order matches the dependency chain (tanh(g) and
    # sig(i) first from PSUM half0 -> i*g; f next -> f*c; o last).
    perm = np.r_[np.arange(2 * H, 3 * H), np.arange(0, H),
                 np.arange(H, 2 * H), np.arange(3 * H, 4 * H)]
    W_ih = W_ih[perm]
    W_hh = W_hh[perm]
    emb = embeddings[captions]                      # [B, T, E] gather
    embT = np.ascontiguousarray(emb.transpose(2, 1, 0)).reshape(E, T * B)
    wihet = np.ascontiguousarray(W_ih[:, :E].T)     # [E, 4H]
    whht = np.ascontiguousarray(
        W_hh.T.reshape(H // 128, 128, 4 * H).transpose(1, 0, 2))
    bias_g = (np.asarray(i["b_ih"]) + np.asarray(i["b_hh"]))[perm]
    featc = (features @ W_ih[:, E:].T + bias_g).astype(np.float32)  # [B, 4H]
    h0 = (features @ np.asarray(i["initH_W"], np.float32)
          + np.asarray(i["initH_b"], np.float32)).astype(np.float32)
    c0 = (features @ np.asarray(i["initC_W"], np.float32)
          + np.asarray(i["initC_b"], np.float32)).astype(np.float32)

    whht2d = whht.reshape(128, -1)
    packr = np.ascontiguousarray(
        np.concatenate([wihet, whht2d, featc], axis=1))
    packf = np.ascontiguousarray(np.concatenate([h0, c0], axis=1))
    in_maps = []
    common = {"embT": embT, "packr": packr, "packf": packf}
    for ci in range(NCORES):
        fcw_c = np.ascontiguousarray(
            fc_W[:, ci * VSH:(ci + 1) * VSH]
            .reshape(H // 128, 128, VP).transpose(1, 0, 2))
        fcb_c = np.ascontiguousarray(
            np.asarray(i["fc_b"], np.float32)[ci * VSH:(ci + 1) * VSH]
            .reshape(1, VP))
        m = dict(common)
        m["fcw"] = fcw_c
        m["fcb"] = fcb_c
        in_maps.append(m)
    return in_maps


def kernel(**inputs):
    global _BUILT, LAST_RESULTS
    in_maps = prepare_in_maps(inputs)

    if _BUILT is None:
        _BUILT = _build_program()
    nc = _BUILT

    res = run_bass_kernel_spmd(nc, in_maps, core_ids=list(range(NCORES)),
                               trace=bool(int(__import__("os").environ.get(
                                   "KERNEL_TRACE", "0"))))
    LAST_RESULTS = res

    out = np.concatenate(
        [res.results[ci]["out"] for ci in range(NCORES)], axis=2)
    return out


# revision 52
# speedup vs baseline: 357.6496x; 1.0261x over previous
"""Trainium2 Bass kernel for nn_DecoderRNN (attention-LSTM decoder, teacher forcing).

Key mathematical simplification: the reference's attention is degenerate --
`feats` has a singleton annotation axis, so softmax over it is exactly 1.0 and
`context == features` at every step. The whole Wa/Ua/va branch cancels out.

What remains per step t:
    gates = emb_t @ W_ih[:, :E].T + features @ W_ih[:, E:].T + b_ih
            + h @ W_hh.T + b_hh
    i,f,g,o = split(gates); c = sig(f)*c + sig(i)*tanh(g); h = sig(o)*tanh(c)
    out[:, t, :] = h @ fc_W + fc_b

Sharding: the output [128, 32, 32000] fp32 (524 MB) dominates; we shard the
vocab dim of fc_W/fc_b/out across the 8 cores (4000 cols each, padded to 4096)
and replicate the (tiny) recurrence on every core. No collectives needed --
each core writes a disjoint V-slice, host concatenates.

Per-core layout (device):
  - batch B=128 lives on SBUF partitions for gates/elementwise/fc-output
  - h is transposed each step ([B,H] -> [H,B] via PE transpose) because both
    the gates matmul and the fc matmul need hT as the stationary operand
  - fc matmul: out[B, 512-slice] = hT.T @ fc_W[hchunk, slice], f32r dtype
    (1 cycle/row vs 4 for fp32)
"""

import numpy as np

import concourse.bass as bass
from concourse import bacc
import concourse.mybir as mybir
import concourse.tile as tile
from concourse.bass import ts
from concourse.bass_utils import run_bass_kernel_spmd
from concourse.masks import make_identity

B, T, V, E, H, F = 128, 32, 32000, 128, 256, 2048
NCORES = 8
VSH = V // NCORES  # 4000 true vocab slice per core
VP = 4000          # per-core fc slice width (8 x 500 chunks)
F32 = mybir.dt.float32
F32R = mybir.dt.float32r

AF = mybir.ActivationFunctionType
OP = mybir.AluOpType

_BUILT = {}  # variant -> compiled program (fc_b==0 skips the bias path)
LAST_RESULTS = None  # BassKernelResults of the last run (for test harness)


KH0 = H // 128


def _build_program(with_fcb=True):
    nc = bacc.Bacc("TRN2", target_bir_lowering=False, debug=False,
                   num_devices=NCORES)

    # ---- DRAM I/O (per-core; all cores share the program, data differs) ----
    d_embT = nc.dram_tensor("embT", [E, T * B], F32R, kind="ExternalInput").ap()
    d_packr = nc.dram_tensor("packr", [128, 4 * H + KH0 * 4 * H + 4 * H], F32R,
                             kind="ExternalInput").ap()
    d_fcw = nc.dram_tensor("fcw", [128, H // 128, VP], F32R,
                           kind="ExternalInput").ap()
    d_packf = nc.dram_tensor("packf", [B, 2 * H], F32, kind="ExternalInput").ap()
    d_fcb = (nc.dram_tensor("fcb", [1, VP], F32, kind="ExternalInput").ap()
             if with_fcb else None)
    d_out = nc.dram_tensor("out", [B, T, VP], F32, kind="ExternalOutput").ap()

    KF = F // 128   # 16 feature-dim chunks
    KH = H // 128   # 2 hidden-dim chunks

    from contextlib import ExitStack

    with tile.TileContext(nc) as tc, ExitStack() as ctx:
        const = ctx.enter_context(tc.tile_pool(name="const", bufs=1))
        state = ctx.enter_context(tc.tile_pool(name="state", bufs=2))
        work = ctx.enter_context(tc.tile_pool(name="work", bufs=2))
        pg = ctx.enter_context(tc.tile_pool(name="pg", bufs=2, space="PSUM"))
        pfc = ctx.enter_context(tc.tile_pool(name="pfc", bufs=6, space="PSUM"))

        ident = const.tile([128, 128], F32)
        make_identity(nc, ident)
        # Warm the ACT function tables (Sigmoid/Tanh/Copy) immediately so
        # their table-load DMAs enqueue before the big input loads -- else
        # step 0's activations stall ~40us behind fcw/embT in the DMA FIFO.
        warm = const.tile([128, 4], F32)
        nc.vector.memset(warm, 0.0)
        nc.scalar.activation(warm[:, 0:1], warm[:, 0:1], AF.Sigmoid)
        nc.scalar.activation(warm[:, 1:2], warm[:, 1:2], AF.Tanh)
        nc.scalar.activation(warm[:, 2:3], warm[:, 2:3], AF.Copy)
        # f32r identity (DVE copy performs the f32r rounding) -- initializes
        # gates PSUM with featc via a single matmul
        identr = const.tile([128, 128], F32R)
        nc.vector.tensor_copy(identr, ident)

        # persistent SBUF tensors. The small recurrence inputs are packed
        # into one [128, 4096] f32r tensor (wihet | whht | featc) and one
        # [128, 512] f32 tensor (h0 | c0) -- each dma_start carries ~1.3us
        # of fixed latency, so fewer/bigger input DMAs shorten the ramp.
        packr_sb = const.tile([128, 4 * H + KH * 4 * H + 4 * H], F32R)
        nc.sync.dma_start(packr_sb, d_packr)
        wihet_sb = packr_sb[:, 0:4 * H]
        def whht_slice(j, hs):
            base = 4 * H + j * 4 * H
            return packr_sb[:, base + hs.start:base + hs.stop]
        featc_sb = packr_sb[:, (1 + KH) * 4 * H:(2 + KH) * 4 * H]
        packf_sb = state.tile([B, 2 * H], F32, tag="packf")
        nc.sync.dma_start(packf_sb, d_packf)
        h_sb = packf_sb[:, 0:H]
        c_sb = packf_sb[:, H:2 * H]
        embT_sb = const.tile([E, T * B], F32R)
        nc.sync.dma_start(embT_sb, d_embT)
        fcw_sb = const.tile([128, KH, VP], F32R)
        nc.sync.dma_start(fcw_sb[:, :, 0:VP // 2], d_fcw[:, :, 0:VP // 2])
        nc.sync.dma_start(fcw_sb[:, :, VP // 2:], d_fcw[:, :, VP // 2:])
        if with_fcb:
            fcb_sb = const.tile([128, VP], F32)
            nc.gpsimd.dma_start(fcb_sb, d_fcb.to_broadcast([128, VP]))
        else:
            fcb_sb = None

        # ---- transpose h -> hT (PSUM slots shared with the fc pool);
        # the two PSUM->SBUF copies run in parallel on DVE and ACT ----
        def transpose_h(h_in):
            hT = state.tile([128, KH, B], F32R, tag="hT", bufs=3)
            pts = []
            for j in range(KH):
                pt = pfc.tile([128, 128], F32, tag="fc", name=f"pt{j}")
                nc.tensor.transpose(pt, h_in[:, j * 128:(j + 1) * 128], ident)
                pts.append(pt)
            nc.vector.tensor_copy(hT[:, 0, :], pts[0])
            nc.vector.tensor_copy(hT[:, 1, :], pts[1])
            return hT

        hT = transpose_h(h_sb)

        # ---- fc half-block (2048 cols): matmuls and moves split so the
        # PSUM->SBUF moves can be issued off the recurrence-critical path ----
        FCC = VP // 8  # 500
        def fc_half_mms(hT_s, half):
            pss = []
            for i in range(4):
                ps = pfc.tile([128, FCC], F32, tag="fc", name=f"fcps{half}_{i}")
                n0 = half * (VP // 2) + i * FCC
                nc.tensor.matmul(ps, hT_s[:, 0, :],
                                 fcw_sb[:, 0, n0:n0 + FCC],
                                 start=True, stop=False)
                nc.tensor.matmul(ps, hT_s[:, 1, :],
                                 fcw_sb[:, 1, n0:n0 + FCC],
                                 start=False, stop=True)
                pss.append(ps)
            return pss

        def fc_half_moves(s, half, pss, on_act):
            # priority-shifted so the scheduler never interleaves these
            # PSUM->SBUF moves into the recurrence-critical ACT/DVE chain
            with tc.high_priority(offset=-100):
                out_sb = work.tile([B, VP // 2], F32, tag=f"fco{half}", bufs=3)
                for i in range(4):
                    n0 = half * (VP // 2) + i * FCC
                    osl = out_sb[:, i * FCC:(i + 1) * FCC]
                    if i in on_act:
                        nc.scalar.activation(osl, pss[i], AF.Copy)
                        if with_fcb:
                            nc.gpsimd.tensor_tensor(
                                osl, osl, fcb_sb[:, n0:n0 + FCC], OP.add)
                    elif with_fcb:
                        nc.vector.tensor_add(osl, pss[i],
                                             fcb_sb[:, n0:n0 + FCC])
                    else:
                        nc.vector.tensor_copy(osl, pss[i])
                nc.sync.dma_start(
                    d_out[:, s, half * (VP // 2):(half + 1) * (VP // 2)],
                    out_sb)

        # ---- the 32 recurrence steps, software-pipelined ----
        # Gate order is host-permuted to [o, g | f, i] (half0 = o,g;
        # half1 = f,i) so tanh(g) can issue right after half0's matmuls.
        # Iteration t works on step t's elementwise chain; the PE queue per
        # iter is [transpose(t), gates-matmuls(t+1), fc(t-1,h1), fc(t,h0)]
        # so the recurrence-critical matmuls never sit behind fc matmuls.
        def emit_gates_mms(t, hT_t):
            g_halves = []
            for hh in range(2):
                hs = slice(hh * 512, (hh + 1) * 512)
                g_ps = pg.tile([128, 512], F32, tag="g", name=f"g_ps{t}_{hh}")
                nc.tensor.matmul(g_ps, identr, featc_sb[:, hs],
                                 start=True, stop=False)
                nc.tensor.matmul(g_ps, embT_sb[:, ts(t, B)],
                                 wihet_sb[:, hs], start=False, stop=False)
                nc.tensor.matmul(g_ps, hT_t[:, 0, :],
                                 whht_slice(0, hs), start=False, stop=False)
                nc.tensor.matmul(g_ps, hT_t[:, 1, :],
                                 whht_slice(1, hs), start=False, stop=True)
                g_halves.append(g_ps)
            return g_halves

        g_halves = emit_gates_mms(0, hT)
        hT_prev = None
        for t in range(T):
            crit = tc.high_priority(offset=None)
            crit.__enter__()
            gates_sb = work.tile([B, 4 * H], F32, tag="gates", bufs=3)
            # activations out of PSUM, gate order [g, i | f, o]:
            # tanh(g) and sig(i) first (both in half0, feed i*g), then f, o
            nc.scalar.activation(gates_sb[:, 0:H], g_halves[0][:, 0:H],
                                 AF.Tanh)
            nc.scalar.activation(gates_sb[:, H:2 * H], g_halves[0][:, H:2 * H],
                                 AF.Sigmoid)
            # c = f*c + i*g ; h = o*tanh(c); i*g first (inputs in half0)
            ig = work.tile([B, H], F32, tag="ig", bufs=3)
            nc.vector.tensor_mul(ig, gates_sb[:, H:2 * H], gates_sb[:, 0:H])
            nc.scalar.activation(gates_sb[:, 2 * H:3 * H],
                                 g_halves[1][:, 0:H], AF.Sigmoid)
            nc.scalar.activation(gates_sb[:, 3 * H:4 * H],
                                 g_halves[1][:, H:2 * H], AF.Sigmoid)
            c_new = state.tile([B, H], F32, tag="c", bufs=3)
            nc.vector.tensor_mul(c_new, gates_sb[:, 2 * H:3 * H], c_sb)
            nc.vector.tensor_add(c_new, c_new, ig)
            tanh_c = work.tile([B, H], F32, tag="th", bufs=3)
            nc.scalar.activation(tanh_c, c_new, AF.Tanh)
            h_new = state.tile([B, H], F32, tag="h", bufs=3)
            nc.vector.tensor_mul(h_new, gates_sb[:, 3 * H:4 * H], tanh_c)
            c_sb = c_new

            hT_prev = hT
            hT = transpose_h(h_new)
            if t + 1 < T:
                g_halves = emit_gates_mms(t + 1, hT)
            crit.__exit__(None, None, None)

            # fc matmuls + moves, all off the critical path:
            # half1 of step t-1 (old hT), then half0 of step t (new hT)
            if t > 0:
                pss_h1 = fc_half_mms(hT_prev, 1)
                fc_half_moves(t - 1, 1, pss_h1, on_act=(0, 1, 2, 3))
            pss_h0 = fc_half_mms(hT, 0)
            fc_half_moves(t, 0, pss_h0, on_act=())

        pss_h1 = fc_half_mms(hT, 1)
        fc_half_moves(T - 1, 1, pss_h1, on_act=(0, 1, 2, 3))

    nc.compile()
    return nc


def prepare_in_maps(inputs):
    i = {k: np.asarray(v) for k, v in inputs.items()}

    features = np.ascontiguousarray(i["features"], dtype=np.float32)
    captions = np.asarray(i["captions"]).astype(np.int64)
    embeddings = np.asarray(i["embeddings"], dtype=np.float32)
    W_ih = np.asarray(i["W_ih"], dtype=np.float32)
    W_hh = np.asarray(i["W_hh"], dtype=np.float32)
    fc_W = np.asarray(i["fc_W"], dtype=np.float32)

    # host-side input prep: sharding, layout transposes, the embedding
    # gather, and the three tiny time-invariant GEMMs (featc, h0, c0 --
    # ~1% of total FLOPs; they would otherwise serialize 13MB of weight
    # DMA in front of step 0 on every core).
    # Gate order is permuted from PyTorch [i,f,g,o] to [g,i,f,o] so the
    # device's activation order matches the dependency chain (tanh(g) and
    # sig(i) first from PSUM half0 -> i*g; f next -> f*c; o last).
    perm = np.r_[np.arange(2 * H, 3 * H), np.arange(0, H),
                 np.arange(H, 2 * H), np.arange(3 * H, 4 * H)]
    W_ih = W_ih[perm]
    W_hh = W_hh[perm]
    emb = embeddings[captions]                      # [B, T, E] gather
    embT = np.ascontiguousarray(emb.transpose(2, 1, 0)).reshape(E, T * B)
    wihet = np.ascontiguousarray(W_ih[:, :E].T)     # [E, 4H]
    whht = np.ascontiguousarray(
        W_hh.T.reshape(H // 128, 128, 4 * H).transpose(1, 0, 2))
    bias_g = (np.asarray(i["b_ih"]) + np.asarray(i["b_hh"]))[perm]
    featc = (features @ W_ih[:, E:].T + bias_g).astype(np.float32)  # [B, 4H]
    h0 = (features @ np.asarray(i["initH_W"], np.float32)
          + np.asarray(i["initH_b"], np.float32)).astype(np.float32)
    c0 = (features @ np.asarray(i["initC_W"], np.float32)
          + np.asarray(i["initC_b"], np.float32)).astype(np.float32)

    whht2d = whht.reshape(128, -1)
    packr = np.ascontiguousarray(
        np.concatenate([wihet, whht2d, featc], axis=1))
    packf = np.ascontiguousarray(np.concatenate([h0, c0], axis=1))
    in_maps = []
    common = {"embT": embT, "packr": packr, "packf": packf}
    for ci in range(NCORES):
        fcw_c = np.ascontiguousarray(
            fc_W[:, ci * VSH:(ci + 1) * VSH]
            .reshape(H // 128, 128, VP).transpose(1, 0, 2))
        fcb_c = np.ascontiguousarray(
            np.asarray(i["fc_b"], np.float32)[ci * VSH:(ci + 1) * VSH]
            .reshape(1, VP))
        m = dict(common)
        m["fcw"] = fcw_c
        m["fcb"] = fcb_c
        in_maps.append(m)
    return in_maps


def kernel(**inputs):
    global LAST_RESULTS
    with_fcb = bool(np.any(np.asarray(inputs["fc_b"])))
    in_maps = prepare_in_maps(inputs)
    if not with_fcb:
        for m in in_maps:
            del m["fcb"]

    if with_fcb not in _BUILT:
        _BUILT[with_fcb] = _build_program(with_fcb)
    nc = _BUILT[with_fcb]

    res = run_bass_kernel_spmd(nc, in_maps, core_ids=list(range(NCORES)),
                               trace=bool(int(__import__("os").environ.get(
                                   "KERNEL_TRACE", "0"))))
    LAST_RESULTS = res

    out = np.concatenate(
        [res.results[ci]["out"] for ci in range(NCORES)], axis=2)
    return out
